# revision 15
# baseline (speedup 1.0000x reference)
"""Distributed Trainium2 kernel for the ActorCritic GNN model.

Strategy: row-shard adj/all_embeds over 8 cores; each GCN layer computes
XW locally ([512,1024]), all-gathers XW (bf16) in two column halves, then
computes h.T = relu(XW_full.T @ adj_rows.T + b) on the tensor engine so
the output is already in the (feature-major) layout the next layer's XW
matmul needs as its stationary operand. Each AllGather half overlaps the
previous half's adj matmuls. The visual branch and the gcn512 head are
K-sharded with a final AllReduce; the tiny MLP head is computed
redundantly on every core.
"""

import numpy as np

R = 8          # cores
N = 4096       # nodes
NR = N // R    # 512 rows per core
F = 1024       # gc feature width
EPAD = 384     # embedding dim padded 300 -> 384 (3 x 128)
KT = N // 128  # 32 k-tiles over nodes

_cache = {}


def _build():
    import concourse.bass as bass
    import concourse.mybir as mybir
    import concourse.tile as tile
    from concourse import bacc
    from concourse.masks import make_identity
    from contextlib import ExitStack

    dt = mybir.dt
    f32, bf16 = dt.float32, dt.bfloat16
    RELU = mybir.ActivationFunctionType.Relu
    COPY = mybir.ActivationFunctionType.Copy
    X = mybir.AxisListType.X

    nc = bacc.Bacc(None, target_bir_lowering=False, num_devices=R,
                   num_swdge_queues=4)

    def param(name, shape):
        return nc.declare_dram_parameter(name, list(shape), f32, isOutput=False)

    adj_rows = param("adj_rows", (NR, N))
    emb_rows = param("emb_rows", (NR, EPAD))
    sem_W = param("sem_W", (512, EPAD))
    sem_b = param("sem_b", (512,))
    word_pad = param("word_pad", (EPAD,))
    scores_pad = param("scores_pad", (1024,))
    score_W = param("score_W", (512, 1024))
    score_b = param("score_b", (512,))
    frames_sl = param("frames_sl", (1024,))
    visual_W_sl = param("visual_W_sl", (512, 1024))
    visual_b = param("visual_b", (512,))
    W1 = param("W1", (F, F))
    b1 = param("b1", (F,))
    W2 = param("W2", (F, F))
    b2 = param("b2", (F,))
    W3 = param("W3", (F, 1))
    b3 = param("b3", (1,))
    g512_W_sl = param("g512_W_sl", (512, NR))
    g512_b = param("g512_b", (512,))
    hid_W = param("hid_W", (512, 1536))
    hid_b = param("hid_b", (512,))
    cri_W = param("cri_W", (1, 512))
    cri_b = param("cri_b", (1,))
    act_W = param("act_W", (6, 512))
    act_b = param("act_b", (6,))
    critic_out = nc.declare_dram_parameter("critic", [1, 1], f32, isOutput=True)
    actor_out = nc.declare_dram_parameter("actor", [1, 6], f32, isOutput=True)

    def bcast_ap(src_ap, parts=128):
        # leading [0, parts] step replicates a flat AP across partitions
        return bass.AP(tensor=src_ap.tensor, offset=src_ap.offset,
                       ap=[[0, parts]] + list(src_ap.ap))

    GRP = [list(range(R))]

    with tile.TileContext(nc) as tc, ExitStack() as ctx:
        const = ctx.enter_context(tc.tile_pool(name="const", bufs=1))
        persist = ctx.enter_context(tc.tile_pool(name="persist", bufs=1))
        dram = ctx.enter_context(tc.tile_pool(name="dram", bufs=1, space="DRAM"))
        psum = ctx.enter_context(tc.tile_pool(name="psum", bufs=8, space="PSUM"))
        lp = ctx.enter_context(tc.tile_pool(name="load", bufs=2))
        adjload = ctx.enter_context(tc.tile_pool(name="adjload", bufs=3))
        xwp = ctx.enter_context(tc.tile_pool(name="xwp", bufs=6))
        w1p = ctx.enter_context(tc.tile_pool(name="w1p", bufs=8))

        def T(pool, shape, dtype, nm, **kw):
            return pool.tile(list(shape), dtype, name=nm, tag=nm, **kw)

        def ps(shape=(128, 512), dtype=f32):
            return T(psum, shape, dtype, "ps")

        # ---------------- DRAM bounce buffers (collectives) ----------------
        xw_in = [[T(dram, [NR, 512], bf16, f"xw{l}_in{h}") for h in range(2)]
                 for l in range(2)]
        xw_out = [[T(dram, [N, 512], bf16, f"xw{l}_out{h}", addr_space="Shared")
                   for h in range(2)] for l in range(2)]
        u_in = T(dram, [1, NR], f32, "u_in")
        u_out = T(dram, [R, NR], f32, "u_out", addr_space="Shared")
        ar_in = T(dram, [1, 1024], f32, "ar_in")
        ar_out = T(dram, [1, 1024], f32, "ar_out", addr_space="Shared")
        h3_rt = T(dram, [1, NR], f32, "h3_rt")

        ident = T(const, [128, 128], f32, "ident")
        make_identity(nc, ident)
        ones_bf = T(const, [1, 128], bf16, "ones")
        nc.gpsimd.memset(ones_bf, 1.0)
        ones_f = T(const, [1, 128], f32, "ones_f")
        nc.gpsimd.memset(ones_f, 1.0)

        # ---------------- persistent SBUF tensors ----------------
        adjT = [T(persist, [128, NR], bf16, f"adjT{k}") for k in range(KT)]
        h1T = [T(persist, [128, NR], bf16, f"h1T{k}") for k in range(8)]
        h2T = [T(persist, [128, NR], bf16, f"h2T{k}") for k in range(8)]
        semWT = [T(persist, [128, 512], bf16, f"semWT{j}") for j in range(3)]
        embT = [T(persist, [128, NR], bf16, f"embT{j}") for j in range(3)]
        emT = [T(persist, [128, NR], bf16, f"emT{m}") for m in range(4)]
        g512WT = [T(persist, [128, 512], bf16, f"g512WT{t}") for t in range(4)]
        hidW_sb = [T(persist, [128, 1536], bf16, f"hidW{t}") for t in range(4)]
        scoresT_bf = T(persist, [128, 4], bf16, "scoresT_bf")
        sc_acc = T(persist, [128, 4], f32, "sc_acc")
        s1_bf = T(persist, [1, F], bf16, "s1_bf")
        vpart = T(persist, [128, 4], f32, "vpart")
        w3_col = T(persist, [128, 8], bf16, "w3_col")
        word_col = T(persist, [128, 3], bf16, "word_col")
        cwT = T(persist, [128, 4], bf16, "cwT")
        awT = T(persist, [128, 4, 6], bf16, "awT")
        # biases, partition-major layouts
        semb_p = T(persist, [128, 4], f32, "semb_p")
        scob_p = T(persist, [128, 4], f32, "scob_p")
        b1_p = T(persist, [128, 8], f32, "b1_p")
        b2_p = T(persist, [128, 8], f32, "b2_p")
        b3_s = T(persist, [1, 1], f32, "b3_s")
        hidb_p = T(persist, [128, 4], f32, "hidb_p")
        # free-dim-layout bias rows used by the head
        semb_row = T(persist, [1, 512], f32, "semb_row")
        visb_row = T(persist, [1, 512], f32, "visb_row")
        g512b_row = T(persist, [1, 512], f32, "g512b_row")
        crib_row = T(persist, [1, 1], f32, "crib_row")
        actb_row = T(persist, [1, 6], f32, "actb_row")
        sem_f = T(persist, [1, 512], f32, "sem_f")
        u_col = T(persist, [128, KT], bf16, "u_col")
        h3_col = T(persist, [128, 4], bf16, "h3_col")

        # ------- loads for the early critical path first (HWDGE, f32) -------
        emb_nat = [T(adjload, [128, EPAD], f32, "emb_nat", bufs=4)
                   for _ in range(4)]
        for i in range(4):
            nc.sync.dma_start(out=emb_nat[i], in_=emb_rows[128 * i:128 * (i + 1), :])

        def scatter_pm(dst, src, tiles):
            # dst[p, t] = src[128*t + p]
            ap = src.ap().rearrange("(t p) -> p t", p=128, t=tiles)
            nc.sync.dma_start(out=dst[:, :tiles], in_=ap)

        scatter_pm(semb_p, sem_b, 4)
        scatter_pm(scob_p, score_b, 4)
        scatter_pm(b1_p, b1, 8)
        scatter_pm(b2_p, b2, 8)
        scatter_pm(hidb_p, hid_b, 4)
        nc.sync.dma_start(out=b3_s, in_=b3.ap()[None, :])
        nc.gpsimd.dma_start(
            out=w3_col, in_=W3.ap().rearrange("(t p) 1 -> p t", p=128, t=8)
        )
        nc.gpsimd.dma_start(
            out=word_col, in_=word_pad.ap().rearrange("(t p) -> p t", p=128, t=3)
        )
        nc.gpsimd.dma_start(
            out=cwT, in_=cri_W.ap().rearrange("1 (t p) -> p t", p=128, t=4)
        )
        for t in range(4):
            nc.gpsimd.dma_start(
                out=awT[:, t, :],
                in_=act_W[:, 128 * t:128 * (t + 1)].rearrange("n p -> p n"),
            )
        nc.sync.dma_start(out=semb_row, in_=sem_b.ap()[None, :])
        nc.sync.dma_start(out=visb_row, in_=visual_b.ap()[None, :])
        nc.sync.dma_start(out=g512b_row, in_=g512_b.ap()[None, :])
        nc.sync.dma_start(out=crib_row, in_=cri_b.ap()[None, :])
        nc.sync.dma_start(out=actb_row, in_=act_b.ap()[None, :])

        # W1/W2 casted loads on the 8 SWDGE queues (needed later)
        w1t = []
        for k in range(8):
            wt = T(w1p, [128, F], bf16, "w1load")
            nc.gpsimd.dma_start(out=wt, in_=W1[128 * k:128 * (k + 1), :])
            w1t.append(wt)
        w2t = []
        for k in range(8):
            wt = T(w1p, [128, F], bf16, "w1load")
            nc.gpsimd.dma_start(out=wt, in_=W2[128 * k:128 * (k + 1), :])
            w2t.append(wt)
        for t in range(4):
            nc.gpsimd.dma_start(out=hidW_sb[t], in_=hid_W[128 * t:128 * (t + 1), :])

        # scores branch on DVE (feeds s1) — broadcast rows via PE outer product
        scores_row = T(persist, [1, 1024], f32, "scores_row")
        nc.sync.dma_start(out=scores_row, in_=scores_pad.ap()[None, :])
        frames_row = T(persist, [1, 1024], f32, "frames_row")
        nc.sync.dma_start(out=frames_row, in_=frames_sl.ap()[None, :])
        scores_bc = T(lp, [128, 1024], f32, "bcast")
        frames_bc = T(lp, [128, 1024], f32, "bcast")
        for n in range(2):
            pb = ps()
            nc.tensor.matmul(pb, ones_f, scores_row[0:1, 512 * n:512 * (n + 1)],
                             start=True, stop=True)
            nc.scalar.activation(scores_bc[:, 512 * n:512 * (n + 1)], pb, COPY)
            pb2 = ps()
            nc.tensor.matmul(pb2, ones_f, frames_row[0:1, 512 * n:512 * (n + 1)],
                             start=True, stop=True)
            nc.scalar.activation(frames_bc[:, 512 * n:512 * (n + 1)], pb2, COPY)
        for t in range(4):
            wt = T(lp, [128, 1024], f32, "brw")
            nc.sync.dma_start(out=wt, in_=score_W[128 * t:128 * (t + 1), :])
            scratch = T(lp, [128, 1024], f32, "scratch", bufs=1)
            nc.vector.tensor_mul(scratch, wt, scores_bc)
            nc.vector.reduce_sum(sc_acc[:, t:t + 1], scratch, axis=X)
        for t in range(4):
            wt = T(lp, [128, 1024], f32, "brw")
            nc.sync.dma_start(out=wt, in_=visual_W_sl[128 * t:128 * (t + 1), :])
            scratch = T(lp, [128, 1024], f32, "scratch", bufs=1)
            nc.vector.tensor_mul(scratch, wt, frames_bc)
            nc.vector.reduce_sum(vpart[:, t:t + 1], scratch, axis=X)
        for t in range(4):
            nc.scalar.activation(
                scoresT_bf[:, t:t + 1], sc_acc[:, t:t + 1], RELU,
                bias=scob_p[:, t:t + 1],
            )


        # ---------------- phase A on PE: emb/semW transposes + em_512.T -----
        def pe_transpose_cast(dst, src_f32):
            tp = ps((128, 128))
            nc.tensor.transpose(tp, src_f32, ident)
            nc.vector.tensor_copy(dst, tp)

        for i in range(4):
            for j in range(3):
                pe_transpose_cast(embT[j][:, 128 * i:128 * (i + 1)],
                                  emb_nat[i][:, 128 * j:128 * (j + 1)])
        for i in range(4):
            snat = T(adjload, [128, EPAD], f32, "emb_nat", bufs=4)
            nc.sync.dma_start(out=snat, in_=sem_W[128 * i:128 * (i + 1), :])
            for j in range(3):
                pe_transpose_cast(semWT[j][:, 128 * i:128 * (i + 1)],
                                  snat[:, 128 * j:128 * (j + 1)])
        for m in range(4):
            pem = ps()
            for j in range(3):
                nc.tensor.matmul(
                    pem, semWT[j][:, 128 * m:128 * (m + 1)], embT[j],
                    start=(j == 0), stop=(j == 2),
                )
            nc.scalar.activation(emT[m], pem, RELU, bias=semb_p[:, m:m + 1])

        # ---------------- XW1 (n-half outer) + AG1 halves ----------------
        # s1 = scores_512 @ W1[:512]  (rank-1 shortcut for the broadcast rows)
        ps_s1 = [ps((1, 512)) for _ in range(2)]
        for k in range(4):
            for n in range(2):
                nc.tensor.matmul(
                    ps_s1[n], scoresT_bf[:, k:k + 1],
                    w1t[k][:, 512 * n:512 * (n + 1)],
                    start=(k == 0), stop=(k == 3),
                )
        for n in range(2):
            nc.scalar.activation(s1_bf[:, 512 * n:512 * (n + 1)], ps_s1[n], COPY)

        for n in range(2):
            ps_h = [ps() for _ in range(4)]
            for k in range(4):
                for m in range(4):
                    nc.tensor.matmul(
                        ps_h[m], emT[k][:, 128 * m:128 * (m + 1)],
                        w1t[4 + k][:, 512 * n:512 * (n + 1)],
                        start=(k == 0), stop=False,
                    )
            for m in range(4):
                nc.tensor.matmul(
                    ps_h[m], ones_bf, s1_bf[:, 512 * n:512 * (n + 1)],
                    start=False, stop=True,
                )
                stg = T(xwp, [128, 512], bf16, "xwstage")
                nc.scalar.activation(stg, ps_h[m], COPY)
                nc.sync.dma_start(
                    out=xw_in[0][n][128 * m:128 * (m + 1), :], in_=stg)
            nc.gpsimd.collective_compute(
                "AllGather", mybir.AluOpType.bypass, replica_groups=GRP,
                ins=[xw_in[0][n].opt()], outs=[xw_out[0][n].opt()],
            )

        # ---------------- adj.T + g512 transposes (overlap AG1) -------------
        for i in range(4):
            for c in range(4):
                anat = T(adjload, [128, 1024], f32, "adj_nat")
                nc.sync.dma_start(
                    out=anat,
                    in_=adj_rows[128 * i:128 * (i + 1), 1024 * c:1024 * (c + 1)],
                )
                for kk in range(8):
                    k = 8 * c + kk
                    pe_transpose_cast(adjT[k][:, 128 * i:128 * (i + 1)],
                                      anat[:, 128 * kk:128 * (kk + 1)])
        for i in range(4):
            gnat = T(adjload, [128, NR], f32, "g512_nat")
            nc.sync.dma_start(out=gnat, in_=g512_W_sl[128 * i:128 * (i + 1), :])
            for t in range(4):
                pe_transpose_cast(g512WT[t][:, 128 * i:128 * (i + 1)],
                                  gnat[:, 128 * t:128 * (t + 1)])

        # semantic branch (free layout, used by the head much later)
        ps_sem = ps((1, 512))
        for j in range(3):
            nc.tensor.matmul(
                ps_sem, word_col[:, j:j + 1], semWT[j], start=(j == 0), stop=(j == 2)
            )
        sem_raw = T(persist, [1, 512], f32, "sem_raw")
        nc.vector.tensor_add(sem_raw, ps_sem, semb_row)
        nc.scalar.activation(sem_f, sem_raw, RELU)

        # ---------------- GCN layers 1 and 2 ----------------
        def gcn_layer(l, bias_p, hT):
            for n in range(2):        # feature half (matches AG half n)
                ps_adj = [ps() for _ in range(4)]
                for k2 in range(KT // 2):
                    xk = T(xwp, [128, 2, 512], bf16, "xk")
                    nc.sync.dma_start(
                        out=xk,
                        in_=xw_out[l][n][256 * k2:256 * (k2 + 1), :].rearrange(
                            "(j p) f -> p j f", p=128),
                    )
                    for j in range(2):
                        k = 2 * k2 + j
                        for m in range(4):
                            nc.tensor.matmul(
                                ps_adj[m], xk[:, j, 128 * m:128 * (m + 1)], adjT[k],
                                start=(k == 0), stop=(k == KT - 1),
                            )
                for m in range(4):
                    mm = 4 * n + m
                    nc.scalar.activation(
                        hT[mm], ps_adj[m], RELU, bias=bias_p[:, mm:mm + 1])

        gcn_layer(0, b1_p, h1T)

        # XW2 = h1 @ W2 (lhsT = h1T), n-half outer, then AG2 halves
        for n in range(2):
            ps_h = [ps() for _ in range(4)]
            for k in range(8):
                for m in range(4):
                    nc.tensor.matmul(
                        ps_h[m], h1T[k][:, 128 * m:128 * (m + 1)],
                        w2t[k][:, 512 * n:512 * (n + 1)],
                        start=(k == 0), stop=(k == 7),
                    )
            for m in range(4):
                stg = T(xwp, [128, 512], bf16, "xwstage")
                nc.scalar.activation(stg, ps_h[m], COPY)
                nc.sync.dma_start(
                    out=xw_in[1][n][128 * m:128 * (m + 1), :], in_=stg)
            nc.gpsimd.collective_compute(
                "AllGather", mybir.AluOpType.bypass, replica_groups=GRP,
                ins=[xw_in[1][n].opt()], outs=[xw_out[1][n].opt()],
            )

        gcn_layer(1, b2_p, h2T)

        # ---------------- layer 3 + gcn512 partial ----------------
        ps_u = ps((1, 512))
        for k in range(8):
            nc.tensor.matmul(
                ps_u, w3_col[:, k:k + 1], h2T[k], start=(k == 0), stop=(k == 7)
            )
        u_sb = T(persist, [1, 512], f32, "u_sb")
        nc.scalar.activation(u_sb, ps_u, COPY)
        nc.sync.dma_start(out=u_in, in_=u_sb)
        nc.gpsimd.collective_compute(
            "AllGather", mybir.AluOpType.bypass, replica_groups=GRP,
            ins=[u_in.opt()], outs=[u_out.opt()],
        )
        nc.gpsimd.dma_start(
            out=u_col, in_=u_out.rearrange("a (t p) -> p (a t)", p=128, t=4)
        )
        ps_h3 = ps((1, 512))
        for k in range(KT):
            nc.tensor.matmul(
                ps_h3, u_col[:, k:k + 1], adjT[k], start=(k == 0), stop=(k == KT - 1)
            )
        h3_sb = T(persist, [1, 512], f32, "h3_sb")
        nc.scalar.activation(h3_sb, ps_h3, RELU, bias=b3_s)
        nc.sync.dma_start(out=h3_rt, in_=h3_sb)
        nc.gpsimd.dma_start(
            out=h3_col, in_=h3_rt.rearrange("a (t p) -> p (a t)", p=128, t=4)
        )
        ps_g = ps((1, 512))
        for t in range(4):
            nc.tensor.matmul(
                ps_g, h3_col[:, t:t + 1], g512WT[t], start=(t == 0), stop=(t == 3)
            )
        gpart_sb = T(persist, [1, 512], f32, "gpart_sb")
        nc.scalar.activation(gpart_sb, ps_g, COPY)

        # ---------------- AllReduce of K-sharded partials ----------------
        for t in range(4):
            nc.sync.dma_start(
                out=ar_in[0:1, 128 * t:128 * (t + 1)], in_=vpart[:, t:t + 1]
            )
        nc.sync.dma_start(out=ar_in[0:1, 512:1024], in_=gpart_sb)
        nc.gpsimd.collective_compute(
            "AllReduce", mybir.AluOpType.add, replica_groups=GRP,
            ins=[ar_in.opt()], outs=[ar_out.opt()],
        )

        # ---------------- head (redundant on every core) ----------------
        vis_raw = T(persist, [1, 512], f32, "vis_raw")
        nc.sync.dma_start(out=vis_raw, in_=ar_out[0:1, 0:512])
        vis_f = T(persist, [1, 512], f32, "vis_f")
        nc.vector.tensor_add(vis_f, vis_raw, visb_row)
        nc.scalar.activation(vis_f, vis_f, RELU)
        gcn_raw = T(persist, [1, 512], f32, "gcn_raw")
        nc.sync.dma_start(out=gcn_raw, in_=ar_out[0:1, 512:1024])
        gcn_f = T(persist, [1, 512], f32, "gcn_f")
        nc.vector.tensor_add(gcn_f, gcn_raw, g512b_row)
        nc.scalar.activation(gcn_f, gcn_f, RELU)

        joint_row = T(persist, [1, 1536], f32, "joint_row")
        nc.vector.tensor_copy(joint_row[0:1, 0:512], vis_f)
        nc.vector.tensor_copy(joint_row[0:1, 512:1024], sem_f)
        nc.vector.tensor_copy(joint_row[0:1, 1024:1536], gcn_f)
        joint_bc = T(persist, [128, 1536], bf16, "joint_bc")
        for j in range(3):
            pb = ps()
            nc.tensor.matmul(pb, ones_f, joint_row[0:1, 512 * j:512 * (j + 1)],
                             start=True, stop=True)
            nc.scalar.activation(joint_bc[:, 512 * j:512 * (j + 1)], pb, COPY)
        xT = T(persist, [128, 4], f32, "xT")
        for t in range(4):
            scratch = T(lp, [128, 1536], bf16, "scratch2")
            nc.vector.tensor_mul(scratch, hidW_sb[t], joint_bc)
            nc.vector.reduce_sum(xT[:, t:t + 1], scratch, axis=X)
        x_bf = T(persist, [128, 4], bf16, "x_bf")
        for t in range(4):
            nc.scalar.activation(
                x_bf[:, t:t + 1], xT[:, t:t + 1], RELU, bias=hidb_p[:, t:t + 1]
            )
        ps_c = ps((1, 1))
        ps_a = ps((1, 6))
        for t in range(4):
            nc.tensor.matmul(
                ps_c, x_bf[:, t:t + 1], cwT[:, t:t + 1], start=(t == 0), stop=(t == 3)
            )
            nc.tensor.matmul(
                ps_a, x_bf[:, t:t + 1], awT[:, t, :], start=(t == 0), stop=(t == 3)
            )
        cri_f = T(persist, [1, 1], f32, "cri_f")
        nc.vector.tensor_add(cri_f, ps_c, crib_row)
        nc.sync.dma_start(out=critic_out[:, :], in_=cri_f)
        act_f = T(persist, [1, 6], f32, "act_f")
        nc.vector.tensor_add(act_f, ps_a, actb_row)
        nc.sync.dma_start(out=actor_out[:, :], in_=act_f)

    nc.compile()
    return nc


def make_in_maps(inputs):
    g = {k: np.asarray(v, dtype=np.float32) for k, v in inputs.items()}
    emb_pad = np.zeros((N, EPAD), np.float32)
    emb_pad[:, :300] = g["all_embeds"]
    semW_pad = np.zeros((512, EPAD), np.float32)
    semW_pad[:, :300] = g["semantic_W"]
    word = np.zeros((EPAD,), np.float32)
    word[:300] = g["word_embed"]
    scores = np.zeros((1024,), np.float32)
    scores[:1000] = g["scores"]
    scoW_pad = np.zeros((512, 1024), np.float32)
    scoW_pad[:, :1000] = g["score_W"]
    frames_flat = g["frames"].reshape(-1)
    common = dict(
        sem_W=semW_pad, sem_b=g["semantic_b"], word_pad=word,
        scores_pad=scores, score_W=scoW_pad, score_b=g["score_b"],
        visual_b=g["visual_b"],
        W1=g["gc1_W"], b1=g["gc1_b"], W2=g["gc2_W"], b2=g["gc2_b"],
        W3=g["gc3_W"], b3=g["gc3_b"], g512_b=g["gcn512_b"],
        hid_W=g["hidden_W"], hid_b=g["hidden_b"],
        cri_W=g["critic_W"], cri_b=g["critic_b"].reshape(1),
        act_W=g["actor_W"], act_b=g["actor_b"],
    )
    in_maps = []
    for r in range(R):
        m = dict(common)
        m["adj_rows"] = np.ascontiguousarray(g["adj"][NR * r:NR * (r + 1), :])
        m["emb_rows"] = np.ascontiguousarray(emb_pad[NR * r:NR * (r + 1), :])
        m["frames_sl"] = np.ascontiguousarray(frames_flat[1024 * r:1024 * (r + 1)])
        m["visual_W_sl"] = np.ascontiguousarray(
            g["visual_W"][:, 1024 * r:1024 * (r + 1)])
        m["g512_W_sl"] = np.ascontiguousarray(
            g["gcn512_W"][:, NR * r:NR * (r + 1)])
        in_maps.append(m)
    return in_maps


def kernel(**inputs):
    from concourse.bass_utils import run_bass_kernel_spmd

    if "nc" not in _cache:
        _cache["nc"] = _build()
    nc = _cache["nc"]
    in_maps = make_in_maps(inputs)
    res = run_bass_kernel_spmd(nc, in_maps, core_ids=list(range(R)))
    out = res.results[0]
    return (np.asarray(out["critic"], np.float32),
            np.asarray(out["actor"], np.float32))


# revision 20
# speedup vs baseline: 1.0913x; 1.0913x over previous
"""Distributed Trainium2 kernel for the ActorCritic GNN model.

Strategy: row-shard adj/all_embeds over 8 cores; each GCN layer computes
XW locally ([512,1024]), all-gathers XW (bf16) in two column halves, then
computes h.T = relu(XW_full.T @ adj_rows.T + b) on the tensor engine so
the output is already in the (feature-major) layout the next layer's XW
matmul needs as its stationary operand. Each AllGather half overlaps the
previous half's adj matmuls. The visual branch and the gcn512 head are
K-sharded with a final AllReduce; the tiny MLP head is computed
redundantly on every core.
"""

import numpy as np

R = 8          # cores
N = 4096       # nodes
NR = N // R    # 512 rows per core
F = 1024       # gc feature width
EPAD = 384     # embedding dim padded 300 -> 384 (3 x 128)
KT = N // 128  # 32 k-tiles over nodes

_cache = {}


def _build():
    import concourse.bass as bass
    import concourse.mybir as mybir
    import concourse.tile as tile
    from concourse import bacc
    from concourse.masks import make_identity
    from contextlib import ExitStack

    dt = mybir.dt
    f32, bf16 = dt.float32, dt.bfloat16
    RELU = mybir.ActivationFunctionType.Relu
    COPY = mybir.ActivationFunctionType.Copy
    X = mybir.AxisListType.X

    nc = bacc.Bacc(None, target_bir_lowering=False, num_devices=R,
                   num_swdge_queues=4)

    def param(name, shape):
        return nc.declare_dram_parameter(name, list(shape), f32, isOutput=False)

    adj_rows = param("adj_rows", (NR, N))
    emb_rows = param("emb_rows", (NR, EPAD))
    sem_W = param("sem_W", (512, EPAD))
    sem_b = param("sem_b", (512,))
    word_pad = param("word_pad", (EPAD,))
    scores_pad = param("scores_pad", (1024,))
    score_W = param("score_W", (512, 1024))
    score_b = param("score_b", (512,))
    frames_sl = param("frames_sl", (1024,))
    visual_W_sl = param("visual_W_sl", (512, 1024))
    visual_b = param("visual_b", (512,))
    W1 = param("W1", (F, F))
    b1 = param("b1", (F,))
    W2 = param("W2", (F, F))
    b2 = param("b2", (F,))
    W3 = param("W3", (F, 1))
    b3 = param("b3", (1,))
    g512_W_sl = param("g512_W_sl", (512, NR))
    g512_b = param("g512_b", (512,))
    hid_W = param("hid_W", (512, 1536))
    hid_b = param("hid_b", (512,))
    cri_W = param("cri_W", (1, 512))
    cri_b = param("cri_b", (1,))
    act_W = param("act_W", (6, 512))
    act_b = param("act_b", (6,))
    critic_out = nc.declare_dram_parameter("critic", [1, 1], f32, isOutput=True)
    actor_out = nc.declare_dram_parameter("actor", [1, 6], f32, isOutput=True)

    def bcast_ap(src_ap, parts=128):
        # leading [0, parts] step replicates a flat AP across partitions
        return bass.AP(tensor=src_ap.tensor, offset=src_ap.offset,
                       ap=[[0, parts]] + list(src_ap.ap))

    GRP = [list(range(R))]

    with tile.TileContext(nc) as tc, ExitStack() as ctx:
        const = ctx.enter_context(tc.tile_pool(name="const", bufs=1))
        persist = ctx.enter_context(tc.tile_pool(name="persist", bufs=1))
        dram = ctx.enter_context(tc.tile_pool(name="dram", bufs=1, space="DRAM"))
        psum = ctx.enter_context(tc.tile_pool(name="psum", bufs=8, space="PSUM"))
        lp = ctx.enter_context(tc.tile_pool(name="load", bufs=2))
        adjload = ctx.enter_context(tc.tile_pool(name="adjload", bufs=3))
        xwp = ctx.enter_context(tc.tile_pool(name="xwp", bufs=6))
        w1p = ctx.enter_context(tc.tile_pool(name="w1p", bufs=8))

        def T(pool, shape, dtype, nm, **kw):
            return pool.tile(list(shape), dtype, name=nm, tag=nm, **kw)

        def ps(shape=(128, 512), dtype=f32):
            return T(psum, shape, dtype, "ps")

        # ---------------- DRAM bounce buffers (collectives) ----------------
        xw_in = [[T(dram, [NR, 512], bf16, f"xw{l}_in{h}") for h in range(2)]
                 for l in range(2)]
        xw_out = [[T(dram, [N, 512], bf16, f"xw{l}_out{h}", addr_space="Shared")
                   for h in range(2)] for l in range(2)]
        u_in = T(dram, [1, NR], f32, "u_in")
        u_out = T(dram, [R, NR], f32, "u_out", addr_space="Shared")
        ar_in = T(dram, [1, 1024], f32, "ar_in")
        ar_out = T(dram, [1, 1024], f32, "ar_out", addr_space="Shared")
        h3_rt = T(dram, [1, NR], f32, "h3_rt")

        # tiny warm-up collective: absorbs the entry barrier + first-collective
        # setup cost while local compute proceeds
        dummy_in = T(dram, [1, 4], f32, "dummy_in")
        dummy_out = T(dram, [R, 4], f32, "dummy_out", addr_space="Shared")
        dummy_sb = T(const, [1, 4], f32, "dummy_sb")
        nc.vector.memset(dummy_sb, 0.0)
        nc.sync.dma_start(out=dummy_in, in_=dummy_sb)
        nc.gpsimd.collective_compute(
            "AllGather", mybir.AluOpType.bypass, replica_groups=GRP,
            ins=[dummy_in.opt()], outs=[dummy_out.opt()],
        )
        ident = T(const, [128, 128], f32, "ident")
        make_identity(nc, ident)
        ones_bf = T(const, [1, 128], bf16, "ones")
        nc.vector.memset(ones_bf, 1.0)
        ones_f = T(const, [1, 128], f32, "ones_f")
        nc.vector.memset(ones_f, 1.0)

        # ---------------- persistent SBUF tensors ----------------
        adjT = [T(persist, [128, NR], bf16, f"adjT{k}") for k in range(KT)]
        h1T = [T(persist, [128, NR], bf16, f"h1T{k}") for k in range(8)]
        h2T = [T(persist, [128, NR], bf16, f"h2T{k}") for k in range(8)]
        semWT = [T(persist, [128, 512], bf16, f"semWT{j}") for j in range(3)]
        embT = [T(persist, [128, NR], bf16, f"embT{j}") for j in range(3)]
        emT = [T(persist, [128, NR], bf16, f"emT{m}") for m in range(4)]
        g512WT = [T(persist, [128, 512], bf16, f"g512WT{t}") for t in range(4)]
        hidW_sb = [T(persist, [128, 1536], bf16, f"hidW{t}") for t in range(4)]
        scoresT_bf = T(persist, [128, 4], bf16, "scoresT_bf")
        sc_acc = T(persist, [128, 4], f32, "sc_acc")
        s1_bf = T(persist, [1, F], bf16, "s1_bf")
        vpart = T(persist, [128, 4], f32, "vpart")
        w3_col = T(persist, [128, 8], bf16, "w3_col")
        word_col = T(persist, [128, 3], bf16, "word_col")
        cwT = T(persist, [128, 4], bf16, "cwT")
        awT = T(persist, [128, 4, 6], bf16, "awT")
        # biases, partition-major layouts
        semb_p = T(persist, [128, 4], f32, "semb_p")
        scob_p = T(persist, [128, 4], f32, "scob_p")
        b1_p = T(persist, [128, 8], f32, "b1_p")
        b2_p = T(persist, [128, 8], f32, "b2_p")
        b3_s = T(persist, [1, 1], f32, "b3_s")
        hidb_p = T(persist, [128, 4], f32, "hidb_p")
        # free-dim-layout bias rows used by the head
        semb_row = T(persist, [1, 512], f32, "semb_row")
        visb_row = T(persist, [1, 512], f32, "visb_row")
        g512b_row = T(persist, [1, 512], f32, "g512b_row")
        crib_row = T(persist, [1, 1], f32, "crib_row")
        actb_row = T(persist, [1, 6], f32, "actb_row")
        sem_f = T(persist, [1, 512], f32, "sem_f")
        u_col = T(persist, [128, KT], bf16, "u_col")
        h3_col = T(persist, [128, 4], bf16, "h3_col")

        # ------- loads for the early critical path first (HWDGE, f32) -------
        def scatter_pm(dst, src, tiles):
            # dst[p, t] = src[128*t + p]
            ap = src.ap().rearrange("(t p) -> p t", p=128, t=tiles)
            nc.sync.dma_start(out=dst[:, :tiles], in_=ap)

        scatter_pm(semb_p, sem_b, 4)
        scatter_pm(scob_p, score_b, 4)
        scatter_pm(b1_p, b1, 8)

        emb_nat = [T(adjload, [128, EPAD], f32, "emb_nat", bufs=4)
                   for _ in range(4)]
        for i in range(4):
            nc.sync.dma_start(out=emb_nat[i], in_=emb_rows[128 * i:128 * (i + 1), :])

        # scores branch on DVE (feeds s1) — broadcast rows via PE outer product
        scores_row = T(persist, [1, 1024], f32, "scores_row")
        nc.sync.dma_start(out=scores_row, in_=scores_pad.ap()[None, :])
        frames_row = T(persist, [1, 1024], f32, "frames_row")
        nc.sync.dma_start(out=frames_row, in_=frames_sl.ap()[None, :])
        scores_bc = T(lp, [128, 1024], f32, "bcast")
        frames_bc = T(lp, [128, 1024], f32, "bcast")
        for n in range(2):
            pb = ps()
            nc.tensor.matmul(pb, ones_f, scores_row[0:1, 512 * n:512 * (n + 1)],
                             start=True, stop=True)
            nc.scalar.activation(scores_bc[:, 512 * n:512 * (n + 1)], pb, COPY)
            pb2 = ps()
            nc.tensor.matmul(pb2, ones_f, frames_row[0:1, 512 * n:512 * (n + 1)],
                             start=True, stop=True)
            nc.scalar.activation(frames_bc[:, 512 * n:512 * (n + 1)], pb2, COPY)
        for t in range(4):
            wt = T(lp, [128, 1024], f32, "brw", bufs=3)
            nc.sync.dma_start(out=wt, in_=score_W[128 * t:128 * (t + 1), :])
            scratch = T(lp, [128, 1024], f32, "scratch", bufs=1)
            nc.vector.tensor_mul(scratch, wt, scores_bc)
            nc.vector.reduce_sum(sc_acc[:, t:t + 1], scratch, axis=X)
        # W1 via HWDGE f32 + DVE cast (keeps gpsimd engine free early)
        w1t = []
        for k in range(8):
            raw = T(lp, [128, F], f32, "brw", bufs=3)
            nc.sync.dma_start(out=raw, in_=W1[128 * k:128 * (k + 1), :])
            wt = T(w1p, [128, F], bf16, "w1load")
            nc.vector.tensor_copy(wt, raw)
            w1t.append(wt)

        for t in range(4):
            wt = T(lp, [128, 1024], f32, "brw", bufs=3)
            nc.sync.dma_start(out=wt, in_=visual_W_sl[128 * t:128 * (t + 1), :])
            scratch = T(lp, [128, 1024], f32, "scratch", bufs=1)
            nc.vector.tensor_mul(scratch, wt, frames_bc)
            nc.vector.reduce_sum(vpart[:, t:t + 1], scratch, axis=X)
        for t in range(4):
            nc.scalar.activation(
                scoresT_bf[:, t:t + 1], sc_acc[:, t:t + 1], RELU,
                bias=scob_p[:, t:t + 1],
            )


        # ---------------- phase A on PE: emb/semW transposes + em_512.T -----
        def pe_transpose_cast(dst, src_f32):
            tp = ps((128, 128))
            nc.tensor.transpose(tp, src_f32, ident)
            nc.vector.tensor_copy(dst, tp)

        for i in range(4):
            for j in range(3):
                pe_transpose_cast(embT[j][:, 128 * i:128 * (i + 1)],
                                  emb_nat[i][:, 128 * j:128 * (j + 1)])
        for i in range(4):
            snat = T(adjload, [128, EPAD], f32, "emb_nat", bufs=4)
            nc.sync.dma_start(out=snat, in_=sem_W[128 * i:128 * (i + 1), :])
            for j in range(3):
                pe_transpose_cast(semWT[j][:, 128 * i:128 * (i + 1)],
                                  snat[:, 128 * j:128 * (j + 1)])
        for m in range(4):
            pem = ps()
            for j in range(3):
                nc.tensor.matmul(
                    pem, semWT[j][:, 128 * m:128 * (m + 1)], embT[j],
                    start=(j == 0), stop=(j == 2),
                )
            nc.scalar.activation(emT[m], pem, RELU, bias=semb_p[:, m:m + 1])

        # ---------------- XW1 (n-half outer) + AG1 halves ----------------
        # s1 = scores_512 @ W1[:512]  (rank-1 shortcut for the broadcast rows)
        ps_s1 = [ps((1, 512)) for _ in range(2)]
        for k in range(4):
            for n in range(2):
                nc.tensor.matmul(
                    ps_s1[n], scoresT_bf[:, k:k + 1],
                    w1t[k][:, 512 * n:512 * (n + 1)],
                    start=(k == 0), stop=(k == 3),
                )
        for n in range(2):
            nc.scalar.activation(s1_bf[:, 512 * n:512 * (n + 1)], ps_s1[n], COPY)

        for n in range(2):
            ps_h = [ps() for _ in range(4)]
            for k in range(4):
                for m in range(4):
                    nc.tensor.matmul(
                        ps_h[m], emT[k][:, 128 * m:128 * (m + 1)],
                        w1t[4 + k][:, 512 * n:512 * (n + 1)],
                        start=(k == 0), stop=False,
                    )
            for m in range(4):
                nc.tensor.matmul(
                    ps_h[m], ones_bf, s1_bf[:, 512 * n:512 * (n + 1)],
                    start=False, stop=True,
                )
                stg = T(xwp, [128, 512], bf16, "xwstage")
                nc.scalar.activation(stg, ps_h[m], COPY)
                nc.sync.dma_start(
                    out=xw_in[0][n][128 * m:128 * (m + 1), :], in_=stg)
            nc.gpsimd.collective_compute(
                "AllGather", mybir.AluOpType.bypass, replica_groups=GRP,
                ins=[xw_in[0][n].opt()], outs=[xw_out[0][n].opt()],
            )

        scatter_pm(b2_p, b2, 8)
        scatter_pm(hidb_p, hid_b, 4)
        nc.sync.dma_start(out=b3_s, in_=b3.ap()[None, :])
        nc.gpsimd.dma_start(
            out=w3_col, in_=W3.ap().rearrange("(t p) 1 -> p t", p=128, t=8)
        )
        nc.gpsimd.dma_start(
            out=word_col, in_=word_pad.ap().rearrange("(t p) -> p t", p=128, t=3)
        )
        nc.gpsimd.dma_start(
            out=cwT, in_=cri_W.ap().rearrange("1 (t p) -> p t", p=128, t=4)
        )
        for t in range(4):
            nc.gpsimd.dma_start(
                out=awT[:, t, :],
                in_=act_W[:, 128 * t:128 * (t + 1)].rearrange("n p -> p n"),
            )
        nc.sync.dma_start(out=semb_row, in_=sem_b.ap()[None, :])
        nc.sync.dma_start(out=visb_row, in_=visual_b.ap()[None, :])
        nc.sync.dma_start(out=g512b_row, in_=g512_b.ap()[None, :])
        nc.sync.dma_start(out=crib_row, in_=cri_b.ap()[None, :])
        nc.sync.dma_start(out=actb_row, in_=act_b.ap()[None, :])

        # W2/hidW casted loads + small scatters (gpsimd engine, after AG1 triggers)
        w2t = []
        for k in range(8):
            wt = T(w1p, [128, F], bf16, "w1load")
            nc.gpsimd.dma_start(out=wt, in_=W2[128 * k:128 * (k + 1), :])
            w2t.append(wt)
        for t in range(4):
            nc.gpsimd.dma_start(out=hidW_sb[t], in_=hid_W[128 * t:128 * (t + 1), :])

        # ---------------- adj.T + g512 transposes (overlap AG1) -------------
        for i in range(4):
            for c in range(4):
                anat = T(adjload, [128, 1024], f32, "adj_nat")
                nc.sync.dma_start(
                    out=anat,
                    in_=adj_rows[128 * i:128 * (i + 1), 1024 * c:1024 * (c + 1)],
                )
                for kk in range(8):
                    k = 8 * c + kk
                    pe_transpose_cast(adjT[k][:, 128 * i:128 * (i + 1)],
                                      anat[:, 128 * kk:128 * (kk + 1)])
        for i in range(4):
            gnat = T(adjload, [128, NR], f32, "g512_nat")
            nc.sync.dma_start(out=gnat, in_=g512_W_sl[128 * i:128 * (i + 1), :])
            for t in range(4):
                pe_transpose_cast(g512WT[t][:, 128 * i:128 * (i + 1)],
                                  gnat[:, 128 * t:128 * (t + 1)])

        # semantic branch (free layout, used by the head much later)
        ps_sem = ps((1, 512))
        for j in range(3):
            nc.tensor.matmul(
                ps_sem, word_col[:, j:j + 1], semWT[j], start=(j == 0), stop=(j == 2)
            )
        sem_raw = T(persist, [1, 512], f32, "sem_raw")
        nc.vector.tensor_add(sem_raw, ps_sem, semb_row)
        nc.scalar.activation(sem_f, sem_raw, RELU)

        # ---------------- GCN layers 1 and 2 ----------------
        def gcn_layer(l, bias_p, hT):
            for n in range(2):        # feature half (matches AG half n)
                ps_adj = [ps() for _ in range(4)]
                for k2 in range(KT // 2):
                    xk = T(xwp, [128, 2, 512], bf16, "xk")
                    nc.sync.dma_start(
                        out=xk,
                        in_=xw_out[l][n][256 * k2:256 * (k2 + 1), :].rearrange(
                            "(j p) f -> p j f", p=128),
                    )
                    for j in range(2):
                        k = 2 * k2 + j
                        for m in range(4):
                            nc.tensor.matmul(
                                ps_adj[m], xk[:, j, 128 * m:128 * (m + 1)], adjT[k],
                                start=(k == 0), stop=(k == KT - 1),
                            )
                for m in range(4):
                    mm = 4 * n + m
                    nc.scalar.activation(
                        hT[mm], ps_adj[m], RELU, bias=bias_p[:, mm:mm + 1])

        gcn_layer(0, b1_p, h1T)

        # XW2 = h1 @ W2 (lhsT = h1T), n-half outer, then AG2 halves
        for n in range(2):
            ps_h = [ps() for _ in range(4)]
            for k in range(8):
                for m in range(4):
                    nc.tensor.matmul(
                        ps_h[m], h1T[k][:, 128 * m:128 * (m + 1)],
                        w2t[k][:, 512 * n:512 * (n + 1)],
                        start=(k == 0), stop=(k == 7),
                    )
            for m in range(4):
                stg = T(xwp, [128, 512], bf16, "xwstage")
                nc.scalar.activation(stg, ps_h[m], COPY)
                nc.sync.dma_start(
                    out=xw_in[1][n][128 * m:128 * (m + 1), :], in_=stg)
            nc.gpsimd.collective_compute(
                "AllGather", mybir.AluOpType.bypass, replica_groups=GRP,
                ins=[xw_in[1][n].opt()], outs=[xw_out[1][n].opt()],
            )

        gcn_layer(1, b2_p, h2T)

        # ---------------- layer 3 + gcn512 partial ----------------
        ps_u = ps((1, 512))
        for k in range(8):
            nc.tensor.matmul(
                ps_u, w3_col[:, k:k + 1], h2T[k], start=(k == 0), stop=(k == 7)
            )
        u_sb = T(persist, [1, 512], f32, "u_sb")
        nc.scalar.activation(u_sb, ps_u, COPY)
        nc.sync.dma_start(out=u_in, in_=u_sb)
        nc.gpsimd.collective_compute(
            "AllGather", mybir.AluOpType.bypass, replica_groups=GRP,
            ins=[u_in.opt()], outs=[u_out.opt()],
        )
        u_col_f = T(lp, [128, KT], f32, "u_col_f", bufs=1)
        nc.sync.dma_start(
            out=u_col_f, in_=u_out.rearrange("a (t p) -> p (a t)", p=128, t=4)
        )
        nc.vector.tensor_copy(u_col, u_col_f)
        ps_h3 = ps((1, 512))
        for k in range(KT):
            nc.tensor.matmul(
                ps_h3, u_col[:, k:k + 1], adjT[k], start=(k == 0), stop=(k == KT - 1)
            )
        h3_sb = T(persist, [1, 512], f32, "h3_sb")
        nc.scalar.activation(h3_sb, ps_h3, RELU, bias=b3_s)
        nc.sync.dma_start(out=h3_rt, in_=h3_sb)
        h3_col_f = T(lp, [128, 4], f32, "h3_col_f", bufs=1)
        nc.sync.dma_start(
            out=h3_col_f, in_=h3_rt.rearrange("a (t p) -> p (a t)", p=128, t=4)
        )
        nc.vector.tensor_copy(h3_col, h3_col_f)
        ps_g = ps((1, 512))
        for t in range(4):
            nc.tensor.matmul(
                ps_g, h3_col[:, t:t + 1], g512WT[t], start=(t == 0), stop=(t == 3)
            )
        gpart_sb = T(persist, [1, 512], f32, "gpart_sb")
        nc.scalar.activation(gpart_sb, ps_g, COPY)

        # ---------------- AllReduce of K-sharded partials ----------------
        for t in range(4):
            nc.sync.dma_start(
                out=ar_in[0:1, 128 * t:128 * (t + 1)], in_=vpart[:, t:t + 1]
            )
        nc.sync.dma_start(out=ar_in[0:1, 512:1024], in_=gpart_sb)
        nc.gpsimd.collective_compute(
            "AllReduce", mybir.AluOpType.add, replica_groups=GRP,
            ins=[ar_in.opt()], outs=[ar_out.opt()],
        )

        # ---------------- head (redundant on every core) ----------------
        vis_raw = T(persist, [1, 512], f32, "vis_raw")
        nc.sync.dma_start(out=vis_raw, in_=ar_out[0:1, 0:512])
        vis_f = T(persist, [1, 512], f32, "vis_f")
        nc.vector.tensor_add(vis_f, vis_raw, visb_row)
        nc.scalar.activation(vis_f, vis_f, RELU)
        gcn_raw = T(persist, [1, 512], f32, "gcn_raw")
        nc.sync.dma_start(out=gcn_raw, in_=ar_out[0:1, 512:1024])
        gcn_f = T(persist, [1, 512], f32, "gcn_f")
        nc.vector.tensor_add(gcn_f, gcn_raw, g512b_row)
        nc.scalar.activation(gcn_f, gcn_f, RELU)

        joint_row = T(persist, [1, 1536], f32, "joint_row")
        nc.vector.tensor_copy(joint_row[0:1, 0:512], vis_f)
        nc.vector.tensor_copy(joint_row[0:1, 512:1024], sem_f)
        nc.vector.tensor_copy(joint_row[0:1, 1024:1536], gcn_f)
        joint_bc = T(persist, [128, 1536], bf16, "joint_bc")
        for j in range(3):
            pb = ps()
            nc.tensor.matmul(pb, ones_f, joint_row[0:1, 512 * j:512 * (j + 1)],
                             start=True, stop=True)
            nc.scalar.activation(joint_bc[:, 512 * j:512 * (j + 1)], pb, COPY)
        xT = T(persist, [128, 4], f32, "xT")
        for t in range(4):
            scratch = T(lp, [128, 1536], bf16, "scratch2")
            nc.vector.tensor_mul(scratch, hidW_sb[t], joint_bc)
            nc.vector.reduce_sum(xT[:, t:t + 1], scratch, axis=X)
        x_bf = T(persist, [128, 4], bf16, "x_bf")
        for t in range(4):
            nc.scalar.activation(
                x_bf[:, t:t + 1], xT[:, t:t + 1], RELU, bias=hidb_p[:, t:t + 1]
            )
        ps_c = ps((1, 1))
        ps_a = ps((1, 6))
        for t in range(4):
            nc.tensor.matmul(
                ps_c, x_bf[:, t:t + 1], cwT[:, t:t + 1], start=(t == 0), stop=(t == 3)
            )
            nc.tensor.matmul(
                ps_a, x_bf[:, t:t + 1], awT[:, t, :], start=(t == 0), stop=(t == 3)
            )
        cri_f = T(persist, [1, 1], f32, "cri_f")
        nc.vector.tensor_add(cri_f, ps_c, crib_row)
        nc.sync.dma_start(out=critic_out[:, :], in_=cri_f)
        act_f = T(persist, [1, 6], f32, "act_f")
        nc.vector.tensor_add(act_f, ps_a, actb_row)
        nc.sync.dma_start(out=actor_out[:, :], in_=act_f)

    nc.compile()
    return nc


def make_in_maps(inputs):
    g = {k: np.asarray(v, dtype=np.float32) for k, v in inputs.items()}
    emb_pad = np.zeros((N, EPAD), np.float32)
    emb_pad[:, :300] = g["all_embeds"]
    semW_pad = np.zeros((512, EPAD), np.float32)
    semW_pad[:, :300] = g["semantic_W"]
    word = np.zeros((EPAD,), np.float32)
    word[:300] = g["word_embed"]
    scores = np.zeros((1024,), np.float32)
    scores[:1000] = g["scores"]
    scoW_pad = np.zeros((512, 1024), np.float32)
    scoW_pad[:, :1000] = g["score_W"]
    frames_flat = g["frames"].reshape(-1)
    common = dict(
        sem_W=semW_pad, sem_b=g["semantic_b"], word_pad=word,
        scores_pad=scores, score_W=scoW_pad, score_b=g["score_b"],
        visual_b=g["visual_b"],
        W1=g["gc1_W"], b1=g["gc1_b"], W2=g["gc2_W"], b2=g["gc2_b"],
        W3=g["gc3_W"], b3=g["gc3_b"], g512_b=g["gcn512_b"],
        hid_W=g["hidden_W"], hid_b=g["hidden_b"],
        cri_W=g["critic_W"], cri_b=g["critic_b"].reshape(1),
        act_W=g["actor_W"], act_b=g["actor_b"],
    )
    in_maps = []
    for r in range(R):
        m = dict(common)
        m["adj_rows"] = np.ascontiguousarray(g["adj"][NR * r:NR * (r + 1), :])
        m["emb_rows"] = np.ascontiguousarray(emb_pad[NR * r:NR * (r + 1), :])
        m["frames_sl"] = np.ascontiguousarray(frames_flat[1024 * r:1024 * (r + 1)])
        m["visual_W_sl"] = np.ascontiguousarray(
            g["visual_W"][:, 1024 * r:1024 * (r + 1)])
        m["g512_W_sl"] = np.ascontiguousarray(
            g["gcn512_W"][:, NR * r:NR * (r + 1)])
        in_maps.append(m)
    return in_maps


def kernel(**inputs):
    from concourse.bass_utils import run_bass_kernel_spmd

    if "nc" not in _cache:
        _cache["nc"] = _build()
    nc = _cache["nc"]
    in_maps = make_in_maps(inputs)
    res = run_bass_kernel_spmd(nc, in_maps, core_ids=list(range(R)))
    out = res.results[0]
    return (np.asarray(out["critic"], np.float32),
            np.asarray(out["actor"], np.float32))
